# revision 22
# baseline (speedup 1.0000x reference)
"""Trainium2 Bass kernel for multi-head cross-attention (dense_transformer).

Reference (per batch element b):
    qh = (q @ w_q)  -> heads [n, h, dk];  kh = (k @ w_k);  vh = (v @ w_v)
    att = softmax(qh @ kh^T * TEMP);  out = (att @ vh) merged @ w_o + q

Distribution: pure data-parallel over batch B=8 across the 8 NeuronCores
(one batch element per core, zero collectives).

Per-core algorithm (all matmuls in bf16 with fp32 PSUM accumulation):
  - cast q/k/v to bf16 in DRAM scratch, DMA-xbar-transpose into SBUF
    (TensorE contracts along the partition axis, so every activation needs
    its contraction dim on partitions).
  - qh^T[hdk, n]  = w_q^T @ q^T   (lhsT = w_q tiles, rhs = q^T)
  - kh^T[hdk, m]  = w_k^T @ k^T
  - vh  [m, hdv]  = v @ w_v       (lhsT = v^T tiles, rhs = w_v)
  - per head: S[n-tile, m-chunk] = qh^T_h.T @ kh^T_h ; E = exp(TEMP*S)
    (no max subtraction: |TEMP*S| <~ 6 for this distribution, exp is safe)
    with per-row accumulation r; E chunks DMA-xbar-transposed to P^T;
    U^T[dv, n] += vh_slice.T @ P^T accumulated over all m.
    Normalize U^T by 1/r broadcast across partitions via a tiny fp32
    matmul against the identity.
  - out[n, dl] = U @ w_o + q  (lhsT = U^T tiles, rhs = w_o), fp32 output.
"""

from contextlib import ExitStack

import numpy as np

import concourse.bass as bass
import concourse.tile as tile
from concourse import bacc, mybir
from concourse.masks import make_identity

F32 = mybir.dt.float32
BF16 = mybir.dt.bfloat16
EXP = mybir.ActivationFunctionType.Exp
MULT = mybir.AluOpType.mult
ADD = mybir.AluOpType.add

B = 8
N = 512          # latent tokens (rows of q)
M = 4096         # byte tokens (rows of k/v)
DL = 1024        # d_latent
DB = 512         # d_byte
H = 8
DK = 128
DV = 128
TEMP = 0.08838834764831845

P = 128          # partitions
NT = N // P      # 4  n-tiles
MC = 512         # m chunk width for S matmuls
NMC = M // MC    # 8  m-chunks
MS = M // P      # 32 m-subtiles
CH = 1024        # rows of k/v per stream chunk
NCH = M // CH    # 4 chunks


def _load_weight_f32_dve(nc, pool, wstage, dst, src_ap, ktiles, width):
    """Plain f32 DMA (full-bandwidth HWDGE) + DVE cast into bf16 dst.

    dst: SBUF tile [P, ktiles, width] bf16; src_ap: DRAM [ktiles*P, width] f32.
    """
    for kt in range(ktiles):
        for ch in range(width // 512):
            st = wstage.tile([P, 512], F32, tag="wstage",
                             name=f"ws_{dst.name}_{kt}_{ch}")
            nc.sync.dma_start(
                out=st,
                in_=src_ap[kt * P:(kt + 1) * P, ch * 512:(ch + 1) * 512])
            nc.vector.tensor_copy(out=dst[:, kt, ch * 512:(ch + 1) * 512], in_=st)


def build_kernel(nc, tc):
    aq = nc.dram_tensor("q", [N, DL], F32, kind="ExternalInput").ap()
    ak = nc.dram_tensor("k", [M, DB], F32, kind="ExternalInput").ap()
    av = nc.dram_tensor("v", [M, DB], F32, kind="ExternalInput").ap()
    awq = nc.dram_tensor("w_q", [DL, H * DK], F32, kind="ExternalInput").ap()
    awk = nc.dram_tensor("w_k", [DB, H * DK], F32, kind="ExternalInput").ap()
    awv = nc.dram_tensor("w_v", [DB, H * DV], F32, kind="ExternalInput").ap()
    awo = nc.dram_tensor("w_o", [H * DV, DL], F32, kind="ExternalInput").ap()
    aout = nc.dram_tensor("out", [N, DL], F32, kind="ExternalOutput").ap()

    with ExitStack() as ctx:
        dram = ctx.enter_context(tc.tile_pool(name="dram", bufs=1, space="DRAM"))
        wpersist = ctx.enter_context(tc.tile_pool(name="wpersist", bufs=1))
        persist = ctx.enter_context(tc.tile_pool(name="persist", bufs=1))
        ps_pool = ctx.enter_context(tc.tile_pool(name="ps", bufs=4, space="PSUM"))
        u_pool = ctx.enter_context(tc.tile_pool(name="psu", bufs=2, space="PSUM"))
        r_pool = ctx.enter_context(tc.tile_pool(name="psr", bufs=2, space="PSUM"))

        # persistent SBUF tensors
        wo_sb = wpersist.tile([P, (H * DV) // P, DL], BF16)      # 16KB/part
        qhT = persist.tile([P, H, N], BF16)                      # 8KB
        kT = persist.tile([P, NCH, DB // P, CH], BF16)           # 32KB  (k^T)
        vh = persist.tile([P, MS, H * DV], BF16)                 # 64KB
        UT = persist.tile([P, H, N], BF16)                       # 8KB

        # DRAM scratch (bf16 copies for xbar transposition)
        k_bf = dram.tile([M, DB], BF16)
        v_bf = dram.tile([M, DB], BF16)

        # ---------- phase Q: load q f32, PE-transpose, project ----------
        ident = wpersist.tile([P, P], F32)
        make_identity(nc, ident)
        wstage = ctx.enter_context(tc.tile_pool(name="wstage", bufs=4))
        with tc.tile_pool(name="qphase", bufs=1) as qpool:
            # SWDGE cast queue carries ONLY q/v/k (cast-DMAs run well below
            # plain-DMA bandwidth; weights go plain-f32 + DVE cast instead)
            q_bf = dram.tile([N, DL], BF16)
            nc.gpsimd.dma_start(out=q_bf, in_=aq)
            for c in range(NCH):
                nc.gpsimd.dma_start(out=v_bf[c * CH:(c + 1) * CH, :],
                                    in_=av[c * CH:(c + 1) * CH, :])
                nc.gpsimd.dma_start(out=k_bf[c * CH:(c + 1) * CH, :],
                                    in_=ak[c * CH:(c + 1) * CH, :])

            wq_sb = qpool.tile([P, DL // P, H * DK], BF16)       # 16KB
            _load_weight_f32_dve(nc, qpool, wstage, wq_sb, awq, DL // P, H * DK)
            qT = qpool.tile([P, DL // P, N], BF16)               # 8KB
            nc.sync.dma_start_transpose(out=qT, in_=q_bf[:])
            wv_sb = wpersist.tile([P, DB // P, H * DV], BF16)     # 8KB
            _load_weight_f32_dve(nc, qpool, wstage, wv_sb, awv, DB // P, H * DV)
            wk_sb = wpersist.tile([P, DB // P, H * DK], BF16)     # 8KB
            _load_weight_f32_dve(nc, qpool, wstage, wk_sb, awk, DB // P, H * DK)
            for h in range(H):
                ps = ps_pool.tile([P, 512], F32, tag="ps")
                for kt in range(DL // P):
                    nc.tensor.matmul(
                        ps[:, :N],
                        lhsT=wq_sb[:, kt, h * DK:(h + 1) * DK],
                        rhs=qT[:, kt, :],
                        start=(kt == 0), stop=(kt == DL // P - 1),
                    )
                nc.vector.tensor_copy(out=qhT[:, h, :], in_=ps[:, :N])

        # ---------- phase V: stream v chunks, project vh; k^T in background --
        with tc.tile_pool(name="stream", bufs=2) as stream:
            for c in range(NCH):
                vT_c = stream.tile([P, DB // P, CH], BF16, tag="stream")
                nc.sync.dma_start_transpose(out=vT_c, in_=v_bf[c * CH:(c + 1) * CH, :])
                # k^T on the same queue (concurrent xbar transposes from two
                # HWDGE queues corrupt data), interleaved after each v chunk
                nc.sync.dma_start_transpose(out=kT[:, c],
                                            in_=k_bf[c * CH:(c + 1) * CH, :])
                for msl in range(CH // P):
                    ms = c * (CH // P) + msl
                    for oc in range(H * DV // 512):
                        ps = ps_pool.tile([P, 512], F32, tag="ps")
                        for kt in range(DB // P):
                            nc.tensor.matmul(
                                ps,
                                lhsT=vT_c[:, kt, msl * P:(msl + 1) * P],
                                rhs=wv_sb[:, kt, oc * 512:(oc + 1) * 512],
                                start=(kt == 0), stop=(kt == DB // P - 1),
                            )
                        nc.vector.tensor_copy(
                            out=vh[:, ms, oc * 512:(oc + 1) * 512], in_=ps)

        # load w_o during attention (DMA is idle by then)
        _load_weight_f32_dve(nc, wpersist, wstage, wo_sb, awo, (H * DV) // P, DL)

        # ---------- attention per head (S^T layout: m on partitions) --------
        # S^T[mt, n] = khT_h_slice.T @ qhT_h ; E^T = exp(TEMP * S^T)
        # U^T[dv, n] += vh_slice.T @ E^T     (contraction over m, no transposes)
        # r[1, n]    += ones.T @ E^T         (softmax denominator)
        # kh projection for head h+1 is woven between S-chunks of head h so
        # the projection phase overlaps the (ACT-paced) attention phase.
        with tc.tile_pool(name="epool", bufs=6) as epool, \
             tc.tile_pool(name="khp", bufs=3) as khp, \
             tc.tile_pool(name="small", bufs=4) as small:
            ones_bf = wpersist.tile([P, 1], BF16)
            nc.vector.memset(ones_bf, 1.0)
            LAG = 3
            NG = NMC  # kh projection groups per head (one per 512-wide chunk)

            def kh_group(khn, hh, g):
                c, mcl = divmod(g, CH // MC)
                ps = ps_pool.tile([P, 512], F32, tag="ps")
                for kt in range(DB // P):
                    nc.tensor.matmul(
                        ps,
                        lhsT=wk_sb[:, kt, hh * DK:(hh + 1) * DK],
                        rhs=kT[:, c, kt, mcl * MC:(mcl + 1) * MC],
                        start=(kt == 0), stop=(kt == DB // P - 1),
                    )
                nc.vector.tensor_copy(out=khn[:, g * MC:(g + 1) * MC], in_=ps)

            kht_cur = khp.tile([P, M], BF16, tag="kh")
            for g in range(NG):
                kh_group(kht_cur, 0, g)

            for h in range(H):
                psU = u_pool.tile([P, N], F32, tag="psu")
                psr = r_pool.tile([1, N], F32, tag="psr")
                ets = [None] * MS
                kht_next = (khp.tile([P, M], BF16, tag="kh", name=f"khn{h}")
                            if h + 1 < H else None)

                def pv_and_rowsum(mt):
                    nc.tensor.matmul(
                        psU,
                        lhsT=vh[:, mt, h * DV:(h + 1) * DV],
                        rhs=ets[mt],
                        start=(mt == 0), stop=(mt == MS - 1),
                    )
                    nc.tensor.matmul(
                        psr,
                        lhsT=ones_bf,
                        rhs=ets[mt],
                        start=(mt == 0), stop=(mt == MS - 1),
                    )

                for mt in range(MS):
                    psS = ps_pool.tile([P, 512], F32, tag="ps")
                    nc.tensor.matmul(
                        psS,
                        lhsT=kht_cur[:, mt * P:(mt + 1) * P],
                        rhs=qhT[:, h, :],
                        start=True, stop=True,
                    )
                    et = epool.tile([P, N], BF16, tag="e")
                    nc.scalar.activation(out=et, in_=psS, func=EXP, scale=TEMP)
                    ets[mt] = et
                    if kht_next is not None and mt % (MS // NG) == 0:
                        kh_group(kht_next, h + 1, mt // (MS // NG))
                    if mt >= LAG:
                        pv_and_rowsum(mt - LAG)
                for mt in range(MS - LAG, MS):
                    pv_and_rowsum(mt)
                kht_cur = kht_next

                # normalize: UT[:, h, :] = psU * (1/r) broadcast over partitions
                # (broadcast via DRAM bounce: SBUF APs need nonzero partition
                # step, DRAM APs don't)
                rec = small.tile([1, N], F32, tag="rec")
                nc.vector.reciprocal(out=rec, in_=psr)
                rec_d = dram.tile([1, N], F32, tag="rec_d")
                nc.sync.dma_start(out=rec_d, in_=rec)
                rbs = small.tile([P, N], F32, tag="rbs")
                nc.sync.dma_start(out=rbs, in_=rec_d.to_broadcast((P, N)))
                nc.vector.tensor_tensor(
                    out=UT[:, h, :], in0=psU, in1=rbs, op=MULT)

        # ---------- output projection + residual ----------
        respool = ctx.enter_context(tc.tile_pool(name="respool", bufs=2))
        for nt in range(NT):
            for oc in range(DL // 512):
                ps = ps_pool.tile([P, 512], F32, tag="ps")
                for kt in range(H * DV // P):
                    nc.tensor.matmul(
                        ps,
                        lhsT=UT[:, kt, nt * P:(nt + 1) * P],
                        rhs=wo_sb[:, kt, oc * 512:(oc + 1) * 512],
                        start=(kt == 0), stop=(kt == H * DV // P - 1),
                    )
                qres = respool.tile([P, 512], F32, tag="qres")
                nc.sync.dma_start(out=qres,
                                  in_=aq[nt * P:(nt + 1) * P, oc * 512:(oc + 1) * 512])
                ot = respool.tile([P, 512], F32, tag="ot")
                nc.vector.tensor_tensor(out=ot, in0=ps, in1=qres, op=ADD)
                nc.sync.dma_start(
                    out=aout[nt * P:(nt + 1) * P, oc * 512:(oc + 1) * 512], in_=ot)


_CACHE = {}


def _get_nc():
    if "nc" not in _CACHE:
        nc = bacc.Bacc("TRN2", target_bir_lowering=False, debug=False)
        with tile.TileContext(nc) as tc:
            build_kernel(nc, tc)
        nc.compile()
        _CACHE["nc"] = nc
    return _CACHE["nc"]


def kernel(q, k, v, w_q, w_k, w_v, w_o):
    from concourse.bass_utils import run_bass_kernel_spmd

    nc = _get_nc()
    in_maps = []
    for i in range(B):
        in_maps.append({
            "q": np.ascontiguousarray(q[i], dtype=np.float32),
            "k": np.ascontiguousarray(k[i], dtype=np.float32),
            "v": np.ascontiguousarray(v[i], dtype=np.float32),
            "w_q": np.ascontiguousarray(w_q, dtype=np.float32),
            "w_k": np.ascontiguousarray(w_k, dtype=np.float32),
            "w_v": np.ascontiguousarray(w_v, dtype=np.float32),
            "w_o": np.ascontiguousarray(w_o, dtype=np.float32),
        })
    res = run_bass_kernel_spmd(nc, in_maps, core_ids=list(range(B)))
    return np.stack([res.results[i]["out"] for i in range(B)], axis=0)


# revision 23
# speedup vs baseline: 1.0315x; 1.0315x over previous
"""Trainium2 Bass kernel for multi-head cross-attention (dense_transformer).

Reference (per batch element b):
    qh = (q @ w_q)  -> heads [n, h, dk];  kh = (k @ w_k);  vh = (v @ w_v)
    att = softmax(qh @ kh^T * TEMP);  out = (att @ vh) merged @ w_o + q

Distribution: pure data-parallel over batch B=8 across the 8 NeuronCores
(one batch element per core, zero collectives).

Per-core algorithm (all matmuls in bf16 with fp32 PSUM accumulation):
  - cast q/k/v to bf16 in DRAM scratch, DMA-xbar-transpose into SBUF
    (TensorE contracts along the partition axis, so every activation needs
    its contraction dim on partitions).
  - qh^T[hdk, n]  = w_q^T @ q^T   (lhsT = w_q tiles, rhs = q^T)
  - kh^T[hdk, m]  = w_k^T @ k^T
  - vh  [m, hdv]  = v @ w_v       (lhsT = v^T tiles, rhs = w_v)
  - per head: S[n-tile, m-chunk] = qh^T_h.T @ kh^T_h ; E = exp(TEMP*S)
    (no max subtraction: |TEMP*S| <~ 6 for this distribution, exp is safe)
    with per-row accumulation r; E chunks DMA-xbar-transposed to P^T;
    U^T[dv, n] += vh_slice.T @ P^T accumulated over all m.
    Normalize U^T by 1/r broadcast across partitions via a tiny fp32
    matmul against the identity.
  - out[n, dl] = U @ w_o + q  (lhsT = U^T tiles, rhs = w_o), fp32 output.
"""

from contextlib import ExitStack

import numpy as np

import concourse.bass as bass
import concourse.tile as tile
from concourse import bacc, mybir
from concourse.masks import make_identity

F32 = mybir.dt.float32
BF16 = mybir.dt.bfloat16
EXP = mybir.ActivationFunctionType.Exp
MULT = mybir.AluOpType.mult
ADD = mybir.AluOpType.add

B = 8
N = 512          # latent tokens (rows of q)
M = 4096         # byte tokens (rows of k/v)
DL = 1024        # d_latent
DB = 512         # d_byte
H = 8
DK = 128
DV = 128
TEMP = 0.08838834764831845

P = 128          # partitions
NT = N // P      # 4  n-tiles
MC = 512         # m chunk width for S matmuls
NMC = M // MC    # 8  m-chunks
MS = M // P      # 32 m-subtiles
CH = 1024        # rows of k/v per stream chunk
NCH = M // CH    # 4 chunks


def _load_weight_f32_dve(nc, pool, wstage, dst, src_ap, ktiles, width):
    """Plain f32 DMA (full-bandwidth HWDGE) + DVE cast into bf16 dst.

    dst: SBUF tile [P, ktiles, width] bf16; src_ap: DRAM [ktiles*P, width] f32.
    """
    for kt in range(ktiles):
        for ch in range(width // 512):
            st = wstage.tile([P, 512], F32, tag="wstage",
                             name=f"ws_{dst.name}_{kt}_{ch}")
            nc.sync.dma_start(
                out=st,
                in_=src_ap[kt * P:(kt + 1) * P, ch * 512:(ch + 1) * 512])
            nc.vector.tensor_copy(out=dst[:, kt, ch * 512:(ch + 1) * 512], in_=st)


def build_kernel(nc, tc):
    aq = nc.dram_tensor("q", [N, DL], F32, kind="ExternalInput").ap()
    ak = nc.dram_tensor("k", [M, DB], F32, kind="ExternalInput").ap()
    av = nc.dram_tensor("v", [M, DB], F32, kind="ExternalInput").ap()
    awq = nc.dram_tensor("w_q", [DL, H * DK], F32, kind="ExternalInput").ap()
    awk = nc.dram_tensor("w_k", [DB, H * DK], F32, kind="ExternalInput").ap()
    awv = nc.dram_tensor("w_v", [DB, H * DV], F32, kind="ExternalInput").ap()
    awo = nc.dram_tensor("w_o", [H * DV, DL], F32, kind="ExternalInput").ap()
    aout = nc.dram_tensor("out", [N, DL], F32, kind="ExternalOutput").ap()

    with ExitStack() as ctx:
        dram = ctx.enter_context(tc.tile_pool(name="dram", bufs=1, space="DRAM"))
        wpersist = ctx.enter_context(tc.tile_pool(name="wpersist", bufs=1))
        persist = ctx.enter_context(tc.tile_pool(name="persist", bufs=1))
        ps_pool = ctx.enter_context(tc.tile_pool(name="ps", bufs=4, space="PSUM"))
        u_pool = ctx.enter_context(tc.tile_pool(name="psu", bufs=2, space="PSUM"))
        r_pool = ctx.enter_context(tc.tile_pool(name="psr", bufs=2, space="PSUM"))

        # persistent SBUF tensors
        wo_sb = wpersist.tile([P, (H * DV) // P, DL], BF16)      # 16KB/part
        qhT = persist.tile([P, H, N], BF16)                      # 8KB
        kT = persist.tile([P, NCH, DB // P, CH], BF16)           # 32KB  (k^T)
        vh = persist.tile([P, MS, H * DV], BF16)                 # 64KB
        UT = persist.tile([P, H, N], BF16)                       # 8KB

        # DRAM scratch (bf16 copies for xbar transposition)
        k_bf = dram.tile([M, DB], BF16)
        v_bf = dram.tile([M, DB], BF16)

        # ---------- phase Q: load q f32, PE-transpose, project ----------
        ident = wpersist.tile([P, P], F32)
        make_identity(nc, ident)
        wstage = ctx.enter_context(tc.tile_pool(name="wstage", bufs=4))
        with tc.tile_pool(name="qphase", bufs=1) as qpool:
            # q: plain f32 load + DVE cast + SBUF->SBUF xbar transposes --
            # completely off the (slow) SWDGE cast queue, so the first
            # matmuls start within ~10us.
            qf = qpool.tile([P, NT, DL], F32)                    # 16KB
            nc.sync.dma_start(out=qf, in_=aq.rearrange("(nt p) d -> p nt d", p=P))
            # SWDGE cast queue carries ONLY v/k
            for c in range(NCH):
                nc.gpsimd.dma_start(out=v_bf[c * CH:(c + 1) * CH, :],
                                    in_=av[c * CH:(c + 1) * CH, :])
                nc.gpsimd.dma_start(out=k_bf[c * CH:(c + 1) * CH, :],
                                    in_=ak[c * CH:(c + 1) * CH, :])

            wq_sb = qpool.tile([P, DL // P, H * DK], BF16)       # 16KB
            _load_weight_f32_dve(nc, qpool, wstage, wq_sb, awq, DL // P, H * DK)
            qb = qpool.tile([P, NT, DL], BF16)                   # 8KB
            for nt_i in range(NT):
                nc.vector.tensor_copy(out=qb[:, nt_i, :], in_=qf[:, nt_i, :])
            # qT[p, nt, j, f] = q[nt*128+f, j*128+p]
            qT = qpool.tile([P, NT, DL // P, P], BF16)           # 8KB
            for nt_i in range(NT):
                nc.sync.dma_start_transpose(out=qT[:, nt_i], in_=qb[:, nt_i, :])
            wv_sb = wpersist.tile([P, DB // P, H * DV], BF16)     # 8KB
            _load_weight_f32_dve(nc, qpool, wstage, wv_sb, awv, DB // P, H * DV)
            wk_sb = wpersist.tile([P, DB // P, H * DK], BF16)     # 8KB
            _load_weight_f32_dve(nc, qpool, wstage, wk_sb, awk, DB // P, H * DK)
            for h in range(H):
                ps = ps_pool.tile([P, 512], F32, tag="ps")
                for kt in range(DL // P):
                    nc.tensor.matmul(
                        ps[:, :N],
                        lhsT=wq_sb[:, kt, h * DK:(h + 1) * DK],
                        rhs=qT[:, :, kt, :],
                        start=(kt == 0), stop=(kt == DL // P - 1),
                    )
                nc.vector.tensor_copy(out=qhT[:, h, :], in_=ps[:, :N])

        # ---------- phase V: stream v chunks, project vh; k^T in background --
        with tc.tile_pool(name="stream", bufs=2) as stream:
            for c in range(NCH):
                vT_c = stream.tile([P, DB // P, CH], BF16, tag="stream")
                nc.sync.dma_start_transpose(out=vT_c, in_=v_bf[c * CH:(c + 1) * CH, :])
                # k^T on the same queue (concurrent xbar transposes from two
                # HWDGE queues corrupt data), interleaved after each v chunk
                nc.sync.dma_start_transpose(out=kT[:, c],
                                            in_=k_bf[c * CH:(c + 1) * CH, :])
                for msl in range(CH // P):
                    ms = c * (CH // P) + msl
                    for oc in range(H * DV // 512):
                        ps = ps_pool.tile([P, 512], F32, tag="ps")
                        for kt in range(DB // P):
                            nc.tensor.matmul(
                                ps,
                                lhsT=vT_c[:, kt, msl * P:(msl + 1) * P],
                                rhs=wv_sb[:, kt, oc * 512:(oc + 1) * 512],
                                start=(kt == 0), stop=(kt == DB // P - 1),
                            )
                        nc.vector.tensor_copy(
                            out=vh[:, ms, oc * 512:(oc + 1) * 512], in_=ps)

        # load w_o during attention (DMA is idle by then)
        _load_weight_f32_dve(nc, wpersist, wstage, wo_sb, awo, (H * DV) // P, DL)

        # ---------- attention per head (S^T layout: m on partitions) --------
        # S^T[mt, n] = khT_h_slice.T @ qhT_h ; E^T = exp(TEMP * S^T)
        # U^T[dv, n] += vh_slice.T @ E^T     (contraction over m, no transposes)
        # r[1, n]    += ones.T @ E^T         (softmax denominator)
        # kh projection for head h+1 is woven between S-chunks of head h so
        # the projection phase overlaps the (ACT-paced) attention phase.
        with tc.tile_pool(name="epool", bufs=6) as epool, \
             tc.tile_pool(name="khp", bufs=3) as khp, \
             tc.tile_pool(name="small", bufs=4) as small:
            ones_bf = wpersist.tile([P, 1], BF16)
            nc.vector.memset(ones_bf, 1.0)
            LAG = 3
            NG = NMC  # kh projection groups per head (one per 512-wide chunk)

            def kh_group(khn, hh, g):
                c, mcl = divmod(g, CH // MC)
                ps = ps_pool.tile([P, 512], F32, tag="ps")
                for kt in range(DB // P):
                    nc.tensor.matmul(
                        ps,
                        lhsT=wk_sb[:, kt, hh * DK:(hh + 1) * DK],
                        rhs=kT[:, c, kt, mcl * MC:(mcl + 1) * MC],
                        start=(kt == 0), stop=(kt == DB // P - 1),
                    )
                nc.vector.tensor_copy(out=khn[:, g * MC:(g + 1) * MC], in_=ps)

            kht_cur = khp.tile([P, M], BF16, tag="kh")
            for g in range(NG):
                kh_group(kht_cur, 0, g)

            for h in range(H):
                psU = u_pool.tile([P, N], F32, tag="psu")
                psr = r_pool.tile([1, N], F32, tag="psr")
                ets = [None] * MS
                kht_next = (khp.tile([P, M], BF16, tag="kh", name=f"khn{h}")
                            if h + 1 < H else None)

                def pv_and_rowsum(mt):
                    nc.tensor.matmul(
                        psU,
                        lhsT=vh[:, mt, h * DV:(h + 1) * DV],
                        rhs=ets[mt],
                        start=(mt == 0), stop=(mt == MS - 1),
                    )
                    nc.tensor.matmul(
                        psr,
                        lhsT=ones_bf,
                        rhs=ets[mt],
                        start=(mt == 0), stop=(mt == MS - 1),
                    )

                for mt in range(MS):
                    psS = ps_pool.tile([P, 512], F32, tag="ps")
                    nc.tensor.matmul(
                        psS,
                        lhsT=kht_cur[:, mt * P:(mt + 1) * P],
                        rhs=qhT[:, h, :],
                        start=True, stop=True,
                    )
                    et = epool.tile([P, N], BF16, tag="e")
                    nc.scalar.activation(out=et, in_=psS, func=EXP, scale=TEMP)
                    ets[mt] = et
                    if kht_next is not None and mt % (MS // NG) == 0:
                        kh_group(kht_next, h + 1, mt // (MS // NG))
                    if mt >= LAG:
                        pv_and_rowsum(mt - LAG)
                for mt in range(MS - LAG, MS):
                    pv_and_rowsum(mt)
                kht_cur = kht_next

                # normalize: UT[:, h, :] = psU * (1/r) broadcast over partitions
                # (broadcast via DRAM bounce: SBUF APs need nonzero partition
                # step, DRAM APs don't)
                rec = small.tile([1, N], F32, tag="rec")
                nc.vector.reciprocal(out=rec, in_=psr)
                rec_d = dram.tile([1, N], F32, tag="rec_d")
                nc.sync.dma_start(out=rec_d, in_=rec)
                rbs = small.tile([P, N], F32, tag="rbs")
                nc.sync.dma_start(out=rbs, in_=rec_d.to_broadcast((P, N)))
                nc.vector.tensor_tensor(
                    out=UT[:, h, :], in0=psU, in1=rbs, op=MULT)

        # ---------- output projection + residual ----------
        respool = ctx.enter_context(tc.tile_pool(name="respool", bufs=2))
        for nt in range(NT):
            for oc in range(DL // 512):
                ps = ps_pool.tile([P, 512], F32, tag="ps")
                for kt in range(H * DV // P):
                    nc.tensor.matmul(
                        ps,
                        lhsT=UT[:, kt, nt * P:(nt + 1) * P],
                        rhs=wo_sb[:, kt, oc * 512:(oc + 1) * 512],
                        start=(kt == 0), stop=(kt == H * DV // P - 1),
                    )
                qres = respool.tile([P, 512], F32, tag="qres")
                nc.sync.dma_start(out=qres,
                                  in_=aq[nt * P:(nt + 1) * P, oc * 512:(oc + 1) * 512])
                ot = respool.tile([P, 512], F32, tag="ot")
                nc.vector.tensor_tensor(out=ot, in0=ps, in1=qres, op=ADD)
                nc.sync.dma_start(
                    out=aout[nt * P:(nt + 1) * P, oc * 512:(oc + 1) * 512], in_=ot)


_CACHE = {}


def _get_nc():
    if "nc" not in _CACHE:
        nc = bacc.Bacc("TRN2", target_bir_lowering=False, debug=False)
        with tile.TileContext(nc) as tc:
            build_kernel(nc, tc)
        nc.compile()
        _CACHE["nc"] = nc
    return _CACHE["nc"]


def kernel(q, k, v, w_q, w_k, w_v, w_o):
    from concourse.bass_utils import run_bass_kernel_spmd

    nc = _get_nc()
    in_maps = []
    for i in range(B):
        in_maps.append({
            "q": np.ascontiguousarray(q[i], dtype=np.float32),
            "k": np.ascontiguousarray(k[i], dtype=np.float32),
            "v": np.ascontiguousarray(v[i], dtype=np.float32),
            "w_q": np.ascontiguousarray(w_q, dtype=np.float32),
            "w_k": np.ascontiguousarray(w_k, dtype=np.float32),
            "w_v": np.ascontiguousarray(w_v, dtype=np.float32),
            "w_o": np.ascontiguousarray(w_o, dtype=np.float32),
        })
    res = run_bass_kernel_spmd(nc, in_maps, core_ids=list(range(B)))
    return np.stack([res.results[i]["out"] for i in range(B)], axis=0)
